# revision 94
# baseline (speedup 1.0000x reference)
import numpy as np

B, N, M, DIN, DH = 8, 2048, 2048, 256, 32
P = 128
QC = 512
NQC = N // QC
NKT = M // P

EXP_A = float(128.0 / np.log(2.0))
EXP_B = float(127.0 * 128.0 - 5.25)
ACT_ROUNDS_BY_CHUNK = {
    0: (0, 2, 4, 6),
    1: (0, 2, 4, 6),
    2: (0, 2, 4, 6),
    3: (0, 2, 4, 6),
}


def _build():
    import concourse.mybir as mybir
    import concourse.tile as tile
    from concourse import bacc
    from concourse.masks import make_identity

    fp32 = mybir.dt.float32
    bf16 = mybir.dt.bfloat16
    i16 = mybir.dt.int16

    nc = bacc.Bacc("TRN2", target_bir_lowering=False, debug=False)

    q_d = nc.dram_tensor("q", [N, DIN], fp32, kind="ExternalInput")
    k_d = nc.dram_tensor("k", [M, DIN], fp32, kind="ExternalInput")
    v_d = nc.dram_tensor("v", [M, DIN], fp32, kind="ExternalInput")
    wq_d = nc.dram_tensor("Wq", [DIN, DH], fp32, kind="ExternalInput")
    wk_d = nc.dram_tensor("Wk", [DIN, DH], fp32, kind="ExternalInput")
    wv_d = nc.dram_tensor("Wv", [DIN, DH], fp32, kind="ExternalInput")
    bq_d = nc.dram_tensor("bq", [DH], fp32, kind="ExternalInput")
    bk_d = nc.dram_tensor("bk", [DH], fp32, kind="ExternalInput")
    bv_d = nc.dram_tensor("bv", [DH], fp32, kind="ExternalInput")
    out_d = nc.dram_tensor("out", [N, DH], fp32, kind="ExternalOutput")

    xdram = {"q": q_d, "k": k_d, "v": v_d}
    wdram = {"q": wq_d, "k": wk_d, "v": wv_d}
    bdram = {"q": bq_d, "k": bk_d, "v": bv_d}

    with tile.TileContext(nc) as tc:
        with (
            tc.tile_pool(name="const", bufs=1) as const,
            tc.tile_pool(name="stage", bufs=1) as stage,
            tc.tile_pool(name="sb", bufs=1) as sb,
            tc.tile_pool(name="expp", bufs=8) as expp,
            tc.tile_pool(name="osb", bufs=2) as osb,
            tc.tile_pool(name="pbig", bufs=3, space="PSUM") as pbig,
            tc.tile_pool(name="po", bufs=2, space="PSUM") as po,
        ):
            wsrc = const.tile([1, 2], bf16)
            nc.gpsimd.memset(wsrc[:], 0.0)
            tdum = const.tile([1, 2], bf16)
            nc.scalar.activation(
                tdum[:], wsrc[:], mybir.ActivationFunctionType.Exp
            )
            id_bf = const.tile([P, P], bf16)

            wf = {}
            bias = {}
            for name in ("q", "k", "v"):
                wfs = const.tile([P, 2, DH], fp32, tag=f"wfs_{name}", name=f"wfs_{name}")
                nc.sync.dma_start(
                    wfs[:], wdram[name][:].rearrange("(o p) c -> p o c", p=P)
                )
                wfb = const.tile([P, 2, DH], bf16, tag=f"wfb_{name}", name=f"wfb_{name}")
                nc.vector.tensor_copy(wfb[:], wfs[:])
                wf[name] = wfb

            for name in ("q", "k"):
                bt = const.tile([DH, 1], fp32, tag=f"b_{name}", name=f"b_{name}")
                nc.scalar.dma_start(
                    bt[:], bdram[name][:].rearrange("(c one) -> c one", one=1)
                )
                bias[name] = bt

            bvs = const.tile([1, DH], fp32)
            nc.scalar.dma_start(
                bvs[:], bdram["v"][:].rearrange("(one c) -> one c", one=1)
            )
            bvb = const.tile([1, DH], bf16)
            nc.vector.tensor_copy(bvb[:], bvs[:])
            ones1 = const.tile([1, P], bf16)
            nc.gpsimd.memset(ones1[:], 1.0)
            zer1 = const.tile([1, 4 * (DH + 1)], bf16)
            nc.gpsimd.memset(zer1[:], 0.0)

            xT = {}
            hT = {}
            for name in ("q", "k"):
                xT[name] = sb.tile([P, 2, N], bf16, tag=f"xT_{name}", name=f"xT_{name}")
                hT[name] = sb.tile([DH, N], bf16, tag=f"hT_{name}", name=f"hT_{name}")
            vT4 = sb.tile([P, 4, 8, P], bf16)
            xk8 = sb.tile([P, 16, P], bf16)
            xq4 = sb.tile([P, 8, P], bf16)
            xq8 = sb.tile([P, 16, P], bf16)
            vh_aug = sb.tile([P, NKT, DH + 1], bf16)
            nc.gpsimd.memset(vh_aug[:, :, DH : DH + 1], 1.0)
            out_sb = sb.tile([P, NKT, DH], fp32)
            out_dst = out_d[:].rearrange("(t p) d -> p t d", p=P)

            xbf = {}
            tmap = {}

            def load(name, t0, nt):
                src = xdram[name][:].rearrange("(t p) d -> p t d", p=P)
                t = stage.tile(
                    [P, nt, DIN], bf16, tag=f"xb_{name}_{t0}", name=f"xb_{name}_{t0}"
                )
                xbf[(name, t0)] = t
                for ts in range(t0, t0 + nt):
                    tmap[(name, ts)] = (t, ts - t0)
                nc.gpsimd.dma_start(t[:], src[:, t0 : t0 + nt, :])

            def transpose_cols(name, t0, nt, engs="vv"):
                for o in range(2):
                    ptp = pbig.tile(
                        [P, nt, P], bf16, tag="big", padded_shape=[P, 8, P]
                    )
                    for i in range(nt):
                        t, li = tmap[(name, t0 + i)]
                        nc.tensor.transpose(
                            ptp[:, i, :], t[:, li, o * P : (o + 1) * P], id_bf[:]
                        )
                    dst = xT[name][:, o, P * t0 : P * (t0 + nt)]
                    if engs[o] == "s":
                        nc.scalar.copy(dst, ptp[:])
                    else:
                        nc.vector.tensor_copy(dst, ptp[:])

            def project_cols(name, t0, nt):
                nb = (nt * P) // QC
                ph = pbig.tile(
                    [DH, nb, QC], fp32, tag="big", padded_shape=[DH, 2, QC]
                )
                for b in range(nb):
                    for o in range(2):
                        nc.tensor.matmul(
                            ph[:, b, :],
                            wf[name][:, o, :],
                            xT[name][:, o, P * t0 + QC * b : P * t0 + QC * (b + 1)],
                            start=(o == 0),
                            stop=(o == 1),
                        )
                nc.scalar.activation(
                    hT[name][:, P * t0 : P * (t0 + nt)].rearrange(
                        "p (a b) -> p a b", b=QC
                    ),
                    ph[:],
                    mybir.ActivationFunctionType.Tanh,
                    bias=bias[name][:],
                )

            def vtrans_pe(g, copy_eng="v"):
                ptp = pbig.tile([P, 8, P], bf16, tag="big")
                for i in range(4):
                    t, li = tmap[("v", 4 * g + i)]
                    for o in range(2):
                        nc.tensor.transpose(
                            ptp[:, 2 * i + o, :],
                            t[:, li, o * P : (o + 1) * P],
                            id_bf[:],
                        )
                if copy_eng == "s":
                    nc.scalar.copy(vT4[:, g], ptp[:])
                else:
                    nc.vector.tensor_copy(vT4[:, g], ptp[:])

            def ktrans8():
                nc.sync.dma_start_transpose(xk8[:], xbf[("k", 8)][:])

            def project_kx(bb):
                rearr = xk8[:].rearrange("p (i two) c -> p two i c", two=2)
                ph = pbig.tile(
                    [DH, 1, QC], fp32, tag="big", padded_shape=[DH, 2, QC]
                )
                for o in range(2):
                    nc.tensor.matmul(
                        ph[:, 0, :],
                        wf["k"][:, o, :],
                        rearr[:, o, 4 * bb : 4 * bb + 4, :],
                        start=(o == 0),
                        stop=(o == 1),
                    )
                nc.scalar.activation(
                    hT["k"][:, 1024 + QC * bb : 1024 + QC * (bb + 1)].rearrange(
                        "p (a b) -> p a b", b=QC
                    ),
                    ph[:],
                    mybir.ActivationFunctionType.Tanh,
                    bias=bias["k"][:],
                )

            def qtrans4():
                nc.sync.dma_start_transpose(xq4[:], xbf[("q", 4)][:, 0:4, :])

            def qtrans8():
                nc.sync.dma_start_transpose(xq8[:], xbf[("q", 4)][:, 4:12, :])

            def project_qx(xq, col0, bb):
                rearr = xq[:].rearrange("p (i two) c -> p two i c", two=2)
                ph = pbig.tile(
                    [DH, 1, QC], fp32, tag="big", padded_shape=[DH, 2, QC]
                )
                for o in range(2):
                    nc.tensor.matmul(
                        ph[:, 0, :],
                        wf["q"][:, o, :],
                        rearr[:, o, 4 * bb : 4 * bb + 4, :],
                        start=(o == 0),
                        stop=(o == 1),
                    )
                nc.scalar.activation(
                    hT["q"][:, col0 : col0 + QC].rearrange(
                        "p (a b) -> p a b", b=QC
                    ),
                    ph[:],
                    mybir.ActivationFunctionType.Tanh,
                    bias=bias["q"][:],
                )

            def vh_fill(g):
                pv = pbig.tile([P, 4, DH], fp32, tag="big")
                for i in range(4):
                    for o in range(2):
                        nc.tensor.matmul(
                            pv[:, i, :],
                            vT4[:, g, 2 * i + o, :],
                            wf["v"][:, o, :],
                            start=(o == 0),
                            stop=False,
                        )
                    nc.tensor.matmul(
                        pv[:, i, :], ones1[:], bvb[:], start=False, stop=True
                    )
                nc.scalar.activation(
                    vh_aug[:, 4 * g : 4 * g + 4, 0:DH],
                    pv[:],
                    mybir.ActivationFunctionType.Tanh,
                )

            state = {"epilogue": None, "po": {}}

            def make_epilogue(c, po_t):
                def epilogue():
                    rec = osb.tile([P, 4, 1], fp32, tag="rec")
                    nc.vector.reciprocal(rec[:], po_t[:, :, DH : DH + 1])
                    nc.vector.tensor_tensor(
                        out_sb[:, 4 * c : 4 * (c + 1), :],
                        po_t[:, :, 0:DH],
                        rec[:, :, :].broadcast_to([P, 4, DH]),
                        mybir.AluOpType.mult,
                    )
                    nc.sync.dma_start(
                        out_dst[:, 4 * c : 4 * (c + 1), :],
                        out_sb[:, 4 * c : 4 * (c + 1), :],
                    )

                return epilogue

            def emit_span(c, r_lo, r_hi, extras=None):
                extras = extras or {}
                qs = slice(QC * c, QC * (c + 1))
                if c not in state["po"]:
                    state["po"][c] = po.tile(
                        [P, 4, DH + 1], fp32, tag="po", name=f"po_{c}"
                    )
                    nc.tensor.matmul(
                        state["po"][c][:],
                        ones1[:],
                        zer1[:],
                        start=True,
                        stop=False,
                        skip_group_check=True,
                    )
                po_t = state["po"][c]
                pTs = {}
                act_rounds = ACT_ROUNDS_BY_CHUNK[c]

                def s_mms(r):
                    pT = pbig.tile([P, 2, QC], fp32, tag="big")
                    pTs[r] = pT
                    for i in range(2):
                        kt = 2 * r + i
                        nc.tensor.matmul(
                            pT[:, i, :],
                            hT["k"][:, P * kt : P * (kt + 1)],
                            hT["q"][:, qs],
                            start=True,
                            stop=True,
                        )

                s_mms(r_lo)
                if r_lo + 1 <= r_hi:
                    s_mms(r_lo + 1)
                for fn in extras.pop(-1, ()):
                    fn()
                if state["epilogue"] is not None:
                    state["epilogue"]()
                    state["epilogue"] = None
                for r in range(r_lo, r_hi + 1):
                    pT = pTs.pop(r)
                    eT = expp.tile([P, 2, QC], bf16, tag="exp")
                    if r in act_rounds:
                        nc.scalar.activation(
                            eT[:], pT[:], mybir.ActivationFunctionType.Exp
                        )
                    else:
                        nc.vector.tensor_scalar(
                            eT[:].bitcast(i16),
                            pT[:],
                            EXP_A,
                            EXP_B,
                            mybir.AluOpType.mult,
                            mybir.AluOpType.add,
                        )
                    if r + 2 <= r_hi:
                        s_mms(r + 2)
                    for fn in extras.pop(r, ()):
                        fn()
                    for i in range(2):
                        kt = 2 * r + i
                        for j in range(4):
                            nc.tensor.matmul(
                                po_t[:, j, :],
                                eT[:, i, P * j : P * (j + 1)],
                                vh_aug[:, kt, :],
                                start=False,
                                stop=(kt == NKT - 1 and j == 3),
                                skip_group_check=True,
                            )
                if r_hi == 7:
                    state["epilogue"] = make_epilogue(c, po_t)

            load("k", 0, 4)
            make_identity(nc, id_bf[:])
            load("q", 0, 4)
            load("k", 4, 4)
            load("v", 0, 4)
            load("v", 4, 4)
            load("k", 8, 8)
            load("v", 8, 8)
            load("q", 4, 12)

            transpose_cols("k", 0, 4)
            project_cols("k", 0, 4)
            transpose_cols("q", 0, 4)
            project_cols("q", 0, 4)
            transpose_cols("k", 4, 4)
            project_cols("k", 4, 4)
            vtrans_pe(0)
            emit_span(
                0, 0, 3,
                extras={
                    -1: [lambda: vh_fill(0)],
                    0: [lambda: vtrans_pe(1)],
                    1: [
                        lambda: vh_fill(1),
                        lambda: transpose_cols("k", 8, 4, "sv"),
                    ],
                    2: [lambda: transpose_cols("k", 12, 4, "sv")],
                    3: [lambda: project_cols("k", 8, 4)],
                },
            )
            project_cols("k", 12, 4)
            vtrans_pe(2)
            qtrans4()
            emit_span(
                0, 4, 7,
                extras={
                    -1: [lambda: vh_fill(2)],
                    4: [
                        lambda: vtrans_pe(3),
                        lambda: project_qx(xq4, 512, 0),
                    ],
                    5: [lambda: vh_fill(3)],
                },
            )
            qtrans8()
            emit_span(
                1, 0, 7,
                extras={
                    1: [lambda: project_qx(xq8, 1024, 0)],
                    3: [lambda: project_qx(xq8, 1536, 1)],
                },
            )
            emit_span(2, 0, 7)
            emit_span(3, 0, 7)
            state["epilogue"]()

    nc.compile()
    return nc


_NC_CACHE = None


def kernel(**inputs) -> np.ndarray:
    global _NC_CACHE
    from concourse.bass_utils import run_bass_kernel_spmd

    if _NC_CACHE is None:
        _NC_CACHE = _build()
    nc = _NC_CACHE

    in_maps = []
    for b in range(B):
        m = {
            "q": np.ascontiguousarray(inputs["q"][b], dtype=np.float32),
            "k": np.ascontiguousarray(inputs["k"][b], dtype=np.float32),
            "v": np.ascontiguousarray(inputs["v"][b], dtype=np.float32),
        }
        for w in ("Wq", "Wk", "Wv", "bq", "bk", "bv"):
            m[w] = np.ascontiguousarray(inputs[w], dtype=np.float32)
        in_maps.append(m)

    res = run_bass_kernel_spmd(nc, in_maps, core_ids=list(range(B)))
    out = np.stack([res.results[b]["out"] for b in range(B)], axis=0)
    return out
